# revision 5
# baseline (speedup 1.0000x reference)
"""Trainium2 Bass kernel for nn_DirectionalDiagram.

out[f, i, j] = X[f, i] + Y[f, j] + x[i, j]        f in [64], i,j in [1024]
  X[f, i] = (cos(t_f) - idx[i]) * 0.5 * cos(t_f)
  Y[f, j] = (sin(t_f) - idx[j]) * 0.5 * sin(t_f)
  idx[i]  = (i - 511.5) / (1024 * sqrt(2))

Sharding: the filter axis is split across the 8 NeuronCores (8 filters per
core); x is replicated.  Per core the kernel is output-bandwidth bound
(32 MiB of f32 writes); the whole computation is a single fused
scalar_tensor_tensor per [128, 1024] output tile:
    out_tile = (x_tile + X_col[f,b]) + YB[f]
where X_col is a per-partition scalar column and YB[f] is Y[f, :]
broadcast across partitions (built once with gpsimd partition_broadcast).
"""

import numpy as np

W = 1024          # image side
P = 128           # SBUF partitions
NB = W // P       # 8 row-blocks
F_TOTAL = 64
N_CORES = 8
F_LOC = F_TOTAL // N_CORES   # 8 filters per core
GH = 4            # row-blocks per output DMA (2 MiB per dma_start)

TRACE = False     # set by test harness to capture an NTFF profile
LAST_RESULT = None

_module_cache = {}


def _build_module():
    import concourse.bacc as bacc
    import concourse.mybir as mybir
    from concourse import tile

    fp32 = mybir.dt.float32
    AOP = mybir.AluOpType

    nc = bacc.Bacc("TRN2", target_bir_lowering=False, debug=False)
    x_d = nc.dram_tensor("x", [W, W], fp32, kind="ExternalInput").ap()
    cs_d = nc.dram_tensor("cs", [2, F_LOC], fp32, kind="ExternalInput").ap()
    idx8_d = nc.dram_tensor("idx8", [F_LOC, W], fp32, kind="ExternalInput").ap()
    idxcol_d = nc.dram_tensor("idxcol", [P, NB], fp32, kind="ExternalInput").ap()
    out_d = nc.dram_tensor("out", [F_LOC, W, W], fp32, kind="ExternalOutput").ap()

    with tile.TileContext(nc) as tc:
        with (
            tc.tile_pool(name="const", bufs=1) as cpool,
            tc.tile_pool(name="outp", bufs=4) as opool,
            tc.tile_pool(name="dscratch", bufs=1, space="DRAM") as dpool,
        ):
            # ---- load x: [1024,1024] -> [128, 8*1024] (block b at cols b*W) ----
            x_sb = cpool.tile([P, NB * W], fp32)
            nc.sync.dma_start(
                out=x_sb[:, :].rearrange("p (b j) -> p b j", j=W),
                in_=x_d.rearrange("(b p) j -> p b j", p=P),
            )

            # ---- tiny inputs ----
            c_row = cpool.tile([1, F_LOC], fp32)
            nc.sync.dma_start(out=c_row[:, :], in_=cs_d[0:1, :])
            s_col = cpool.tile([F_LOC, 1], fp32)
            nc.sync.dma_start(out=s_col[:, :], in_=cs_d[1:2, :].transpose([1, 0]))
            idx8_sb = cpool.tile([F_LOC, W], fp32)
            nc.sync.dma_start(out=idx8_sb[:, :], in_=idx8_d[:, :])
            idxcol_sb = cpool.tile([P, NB], fp32)
            nc.sync.dma_start(out=idxcol_sb[:, :], in_=idxcol_d[:, :])

            # ---- Y rows: y_loc[f, j] = (idx[j] - s[f]) * (-0.5*s[f]) ----
            sm_col = cpool.tile([F_LOC, 1], fp32)
            nc.vector.tensor_scalar_mul(sm_col[:, :], s_col[:, :], -0.5)
            y_loc = cpool.tile([F_LOC, W], fp32)
            nc.vector.tensor_scalar(
                y_loc[:, :],
                idx8_sb[:, :],
                s_col[:, 0:1],
                sm_col[:, 0:1],
                AOP.subtract,
                AOP.mult,
            )

            # ---- YB[f] = Y[f, :] broadcast to all 128 partitions ----
            # partition_broadcast needs its source on partition 0, so first
            # flatten y_loc's 8 partition-rows into one row via DRAM scratch.
            ysc = dpool.tile([F_LOC, W], fp32)
            nc.sync.dma_start(out=ysc[:, :], in_=y_loc[:, :])
            y_rows = cpool.tile([1, F_LOC * W], fp32)
            nc.sync.dma_start(
                out=y_rows[:, :], in_=ysc[:, :].flatten().unsqueeze(0)
            )
            yb = cpool.tile([P, F_LOC * W], fp32)
            nc.gpsimd.partition_broadcast(yb[:, :], y_rows[:, :])

            # ---- X columns: xc[p, f*NB+b] = (c[f] - idx[b*128+p]) * 0.5*c[f] ----
            cB = cpool.tile([P, F_LOC], fp32)
            nc.gpsimd.partition_broadcast(cB[:, :], c_row[:, :])
            t1 = cpool.tile([P, F_LOC * NB], fp32)
            nc.vector.tensor_tensor(
                t1[:, :].rearrange("p (f b) -> p f b", b=NB),
                cB[:, :].unsqueeze(2).broadcast_to([P, F_LOC, NB]),
                idxcol_sb[:, :].unsqueeze(1).broadcast_to([P, F_LOC, NB]),
                AOP.subtract,
            )
            ch = cpool.tile([P, F_LOC], fp32)
            nc.vector.tensor_scalar_mul(ch[:, :], cB[:, :], 0.5)
            xc = cpool.tile([P, F_LOC * NB], fp32)
            nc.vector.tensor_tensor(
                xc[:, :].rearrange("p (f b) -> p f b", b=NB),
                t1[:, :].rearrange("p (f b) -> p f b", b=NB),
                ch[:, :].unsqueeze(2).broadcast_to([P, F_LOC, NB]),
                AOP.mult,
            )

            # ---- main loop: one fused op per [128, 1024] output tile ----
            out_r = out_d.rearrange("f (g p) j -> f p g j", p=P)
            for f in range(F_LOC):
                for h in range(NB // GH):
                    big = opool.tile([P, GH * W], fp32, tag="big")
                    for k in range(GH):
                        b = h * GH + k
                        q = f * NB + b
                        nc.vector.scalar_tensor_tensor(
                            big[:, k * W : (k + 1) * W],
                            x_sb[:, b * W : (b + 1) * W],
                            xc[:, q : q + 1],
                            yb[:, f * W : (f + 1) * W],
                            AOP.add,
                            AOP.add,
                        )
                    nc.sync.dma_start(
                        out=out_r[f, :, h * GH : (h + 1) * GH, :],
                        in_=big[:, :].rearrange("p (g j) -> p g j", j=W),
                    )
    nc.compile()
    return nc


def _get_module():
    if "nc" not in _module_cache:
        _module_cache["nc"] = _build_module()
    return _module_cache["nc"]


def _host_inputs(x, filters):
    x = np.ascontiguousarray(x, dtype=np.float32)
    filters = np.asarray(filters, dtype=np.float32).reshape(F_TOTAL)
    c = np.cos(filters)
    s = np.sin(filters)
    denom = np.float32(W) * np.sqrt(np.float32(2.0))
    idx = (np.arange(W, dtype=np.float32) - np.float32(W / 2 - 0.5)) / denom
    idx8 = np.ascontiguousarray(np.broadcast_to(idx, (F_LOC, W)))
    idxcol = np.ascontiguousarray(idx.reshape(NB, P).T)  # [128, 8]
    in_maps = []
    for core in range(N_CORES):
        sl = slice(core * F_LOC, (core + 1) * F_LOC)
        in_maps.append(
            {
                "x": x,
                "cs": np.ascontiguousarray(np.stack([c[sl], s[sl]])),
                "idx8": idx8,
                "idxcol": idxcol,
            }
        )
    return in_maps


def kernel(x, filters):
    global LAST_RESULT
    import concourse.bass_utils as bass_utils

    nc = _get_module()
    in_maps = _host_inputs(x, filters)
    res = bass_utils.run_bass_kernel_spmd(
        nc,
        in_maps,
        core_ids=list(range(N_CORES)),
        trace=TRACE,
        stitch_traces=False,
    )
    LAST_RESULT = res
    return np.concatenate([r["out"] for r in res.results], axis=0)
